# revision 23
# baseline (speedup 1.0000x reference)
"""Multi-head self-attention on 8 Trainium2 NeuronCores.

Tensor-parallel over heads: core c owns heads 2c, 2c+1 (128 of the 1024
hidden columns).  The host pre-transposes x to x^T [1024, 4096] bf16 and
the per-core weight slices to [p, k*c] layout so every DMA is contiguous
2KB-per-partition lines.  Stages:
  1. Q^T/K^T = (w.T @ x^T + b) in [d, token] layout (2 heads stacked on
     partitions: 0:64 head0, 64:128 head1).
  2. V^T likewise, then PE transposes into V_aug [token, 65-per-head]
     where column 64/129 = 1.0 (ones column -> softmax denominator falls
     out of P@V as accumulator row 64).
  3. Attention in 4 chunks of (batch, 1024 queries), software-pipelined:
     scores^T tiles = K^T.T @ Q^T (K=64 contraction; the two heads run
     concurrently in disjoint PE row groups), P^T = exp(S^T/8) on ScalarE
     (|S/8| < 3 so exp cannot overflow), and the previous chunk's P@V
     accumulation plus projection/WO back-work fill the PE while ScalarE
     (the bottleneck, ~1.1us per [128,1024] exp) streams.
  4. normalize: rsum = recip(pso row 64) straight from PSUM, gpsimd
     partition_broadcast, one fused tensor_mul PSUM->attnT (bf16).
  5. partial = attnT.T @ wo[128 rows of this core] -> HBM (bf16), the
     two 512-col halves staged into one [128,1024] tile, single DMA.
Host sums the 8 partials and adds bo.

Scheduling: ScalarE must never starve.  Warm-up matmuls + the exp
ACT-table load issue at t=0 with no DMA dependency (HAM un-throttles the
PE clock during the DMA window and the 2.7us table load is off the
critical path).  x^T lands in first-use order; weights go on the
tensor/vector DMA queues so they never queue behind x^T.  The extras
(projection chunks, V build, WO tiles) are spread so each chunk's PE
work stays under the ~35.7us exp stream; scores for step tt+1 (and for
the next chunk's tt=0 at boundaries) are emitted before slower PE work.

Shapes hardcoded for x:[2,2048,1024], 16 heads, d_k=64.
"""

import numpy as np
import ml_dtypes

import concourse.bass as bass
import concourse.tile as tile
from concourse import bacc, mybir
from concourse.bass import ts
from concourse.bass_utils import run_bass_kernel_spmd

BF16 = mybir.dt.bfloat16
F32 = mybir.dt.float32
NPBF16 = ml_dtypes.bfloat16

B = 2
S = 2048
D = 1024
NT = B * S  # 4096 tokens
DK = 64
NCORES = 8
HPC = 2  # heads per core
SC = 1024  # attention s-chunk (exp op free size)

_CACHE = {}


def _build_nc():
    nc = bacc.Bacc("TRN2", target_bir_lowering=False, debug=False,
                   num_devices=NCORES)

    xT = nc.dram_tensor("xT", [D, NT], BF16, kind="ExternalInput").ap()
    # weights host-transposed to [p, k*128] so the DMA is contiguous
    wq = nc.dram_tensor("wq", [128, D], BF16, kind="ExternalInput").ap()
    wk = nc.dram_tensor("wk", [128, D], BF16, kind="ExternalInput").ap()
    wv = nc.dram_tensor("wv", [128, D], BF16, kind="ExternalInput").ap()
    bqkv = nc.dram_tensor("bqkv", [128, 3], F32, kind="ExternalInput").ap()
    wo = nc.dram_tensor("wo", [128, D], BF16, kind="ExternalInput").ap()
    out = nc.dram_tensor("out", [NT, D], BF16, kind="ExternalOutput").ap()

    with tile.TileContext(nc) as tc:
        _emit(nc, tc, xT, wq, wk, wv, bqkv, wo, out)
    nc.compile()
    return nc


def _emit(nc, tc, xT, wq, wk, wv, bqkv, wo, out):
    import contextlib
    ctx = contextlib.ExitStack()
    with ctx:
        consts = ctx.enter_context(tc.tile_pool(name="consts", bufs=1))
        ptp = ctx.enter_context(tc.tile_pool(name="ptp", bufs=46))
        psp = ctx.enter_context(tc.tile_pool(name="psp", bufs=2, space="PSUM"))
        pvp = ctx.enter_context(tc.tile_pool(name="pvp", bufs=2, space="PSUM"))
        pse = ctx.enter_context(tc.tile_pool(name="pse", bufs=2, space="PSUM"))
        stg = ctx.enter_context(tc.tile_pool(name="stg", bufs=2))
        nrm = ctx.enter_context(tc.tile_pool(name="nrm", bufs=2))
        vstg = ctx.enter_context(tc.tile_pool(name="vstg", bufs=2))

        # ---- persistent SBUF tensors ----
        xT_sb = consts.tile([128, 8, NT], BF16)      # 8 k-tiles of x^T
        wq_sb = consts.tile([128, 8, 128], BF16)
        wk_sb = consts.tile([128, 8, 128], BF16)
        wv_sb = consts.tile([128, 8, 128], BF16)
        bqkv_sb = consts.tile([128, 3], F32)
        bq_sb = bqkv_sb[:, 0:1]
        bk_sb = bqkv_sb[:, 1:2]
        bv_sb = bqkv_sb[:, 2:3]
        wo_sb = consts.tile([128, D], BF16)
        QT = consts.tile([128, NT], BF16)
        KT = consts.tile([128, NT], BF16)
        V_sb = consts.tile([128, 32, 130], BF16)     # [t-in-tile, t_tile, col]
        attnT = consts.tile([128, NT], BF16)
        ident = consts.tile([128, 128], BF16)
        wup = consts.tile([128, 128], BF16)

        # ---- t=0: PE warm-up + ACT table load, no DMA dependencies ----
        nc.vector.memset(wup, 0.125)
        wups = pse.tile([128, 512], F32, tag="pse", name="wups")
        for i in range(64):
            nc.tensor.matmul(wups[:, 0:128], lhsT=wup, rhs=wup,
                             start=True, stop=True)
        tblw = stg.tile([128, 1], F32, tag="tblw", name="tblw")
        nc.scalar.activation(out=tblw, in_=wup[:, 0:1],
                             func=mybir.ActivationFunctionType.Exp)
        from concourse.masks import make_identity
        make_identity(nc, ident)

        # ---- DMAs: weights first on the gpsimd queue; x^T in merged
        # multi-k-tile waves (one dma_start covers all 8 k-tiles of a
        # column range -- 8x fewer descriptor-gen instructions).  The
        # scalar (ACT) queue carries exactly one wave so the exp stream
        # is never blocked behind descriptor generation. ----
        xT_p = xT.rearrange("(k p) n -> p k n", p=128)

        # critical first: x^T[:, 0:1024] (2MB) split across the sync and
        # scalar queues, weights in parallel on gpsimd.  The rest follows
        # once the critical slices are in flight.
        # the critical x^T[:, 0:1024] is split across ALL THREE queues
        # (per-queue bandwidth is only ~130 B/ns); small weights slot in
        # by first-need time.
        nc.sync.dma_start(out=xT_sb[:, 0:3, 0:1024], in_=xT_p[:, 0:3, 0:1024])
        nc.scalar.dma_start(out=xT_sb[:, 3:6, 0:1024],
                            in_=xT_p[:, 3:6, 0:1024])
        nc.gpsimd.dma_start(out=xT_sb[:, 6:8, 0:1024],
                            in_=xT_p[:, 6:8, 0:1024])
        nc.gpsimd.dma_start(out=wq_sb, in_=wq.rearrange("p (k c) -> p k c", k=8))
        nc.gpsimd.dma_start(out=bqkv_sb, in_=bqkv)
        nc.gpsimd.dma_start(out=wk_sb, in_=wk.rearrange("p (k c) -> p k c", k=8))
        nc.scalar.dma_start(out=wv_sb, in_=wv.rearrange("p (k c) -> p k c", k=8))
        nc.sync.dma_start(out=xT_sb[:, :, 1024:1536],
                          in_=xT_p[:, :, 1024:1536])
        nc.gpsimd.dma_start(out=xT_sb[:, :, 1536:2048],
                            in_=xT_p[:, :, 1536:2048])
        nc.sync.dma_start(out=xT_sb[:, :, 2048:3072],
                          in_=xT_p[:, :, 2048:3072])
        nc.gpsimd.dma_start(out=wo_sb, in_=wo)
        nc.gpsimd.dma_start(out=xT_sb[:, :, 3072:4096],
                            in_=xT_p[:, :, 3072:4096])

        # ones columns of V_aug (never touched by the per-tile copies)
        nc.vector.memset(V_sb[:, :, 64:65], 1.0)
        nc.vector.memset(V_sb[:, :, 129:130], 1.0)

        # ---- emit helpers ----
        vt_stage = {}

        def emit_v_proj(c):
            # V^T chunk: [c128, 512 tokens] += wv[k].T @ xT[k] (+bias, ->bf16)
            psv = pse.tile([128, 512], F32, tag="pse")
            for k in range(8):
                nc.tensor.matmul(psv, lhsT=wv_sb[:, k, :],
                                 rhs=xT_sb[:, k, ts(c, 512)],
                                 start=(k == 0), stop=(k == 7))
            vt = vstg.tile([128, 512], BF16, tag="vt", name=f"vt{c}")
            nc.vector.tensor_scalar_add(vt, psv, bv_sb)
            vt_stage[c] = vt

        def emit_v_tr(tt):
            # transpose one 128x128 block of V^T into V_aug [t, col] layout
            c, j = divmod(tt, 4)
            trp = pse.tile([128, 128], BF16, tag="pse", name=f"trp{tt}")
            nc.tensor.transpose(trp, vt_stage[c][:, ts(j, 128)], ident)
            nc.vector.tensor_copy(V_sb[:, tt, 0:64], trp[:, 0:64])
            nc.vector.tensor_copy(V_sb[:, tt, 65:129], trp[:, 64:128])

        def emit_wo_tile(tt, use_act=False):
            for eh in range(2):
                pw = pse.tile([128, 512], F32, tag="pse")
                nc.tensor.matmul(pw, lhsT=attnT[:, ts(tt, 128)],
                                 rhs=wo_sb[:, ts(eh, 512)],
                                 start=True, stop=True)
                ob = stg.tile([128, 512], BF16, tag="ob")
                if use_act and eh == 1:
                    nc.scalar.activation(
                        out=ob, in_=pw,
                        func=mybir.ActivationFunctionType.Copy, bias=0.0)
                else:
                    nc.vector.tensor_copy(ob, pw)
                nc.sync.dma_start(
                    out=out[tt * 128:(tt + 1) * 128, eh * 512:(eh + 1) * 512],
                    in_=ob)

        def emit_proj_chunk(w_sb, b_sb, o_sb, n, w=512):
            # w-token chunk n (units of w) of the Q^T or K^T projection
            ps = pse.tile([128, 512], F32, tag="pse")
            for k in range(8):
                nc.tensor.matmul(ps[:, 0:w], lhsT=w_sb[:, k, :],
                                 rhs=xT_sb[:, k, ts(n, w)],
                                 start=(k == 0), stop=(k == 7))
            nc.vector.tensor_scalar_add(o_sb[:, ts(n, w)], ps[:, 0:w], b_sb)

        def emit_normalize_half(prev, c):
            # stage the accumulator out of PSUM first (one copy frees the
            # bank for the next PV half).  reciprocal_approx_fast is a
            # custom-DVE op: it needs a partition-0 SBUF input (PSUM or
            # offset-64 inputs produce garbage / hw crashes).
            b, sc, pts, pv_state = prev
            s0 = b * S + sc * SC + c * 512
            for h in range(HPC):
                pso = pv_state['psos'][h]
                ostg = nrm.tile([65, 512], F32, tag="ostg", bufs=1)
                nc.vector.tensor_copy(ostg, pso[0:65, :])
                rsum = nrm.tile([1, 512], F32, tag="rsum", bufs=1)
                nc.vector.tensor_copy(rsum, ostg[64:65, :])
                nc.vector.reciprocal_approx_fast(out=rsum, in_=rsum)
                recb = nrm.tile([64, 512], F32, tag="recb", bufs=1)
                nc.gpsimd.partition_broadcast(recb, rsum)
                nc.vector.tensor_mul(
                    attnT[h * DK:(h + 1) * DK, s0:s0 + 512],
                    ostg[0:64, :], recb)

        def emit_pv_step(prev, s):
            # one pipeline step of P@V for the previous chunk: two t-tiles
            # into the [65, 512] accumulators of half-chunk c = s // 8
            b, sc, pts, pv_state = prev
            c = s // 8
            if s % 8 == 0:
                pv_state['psos'] = [
                    pvp.tile([128, 512], F32, tag="pv",
                             name=f"pso{b}_{sc}_{c}_{h}")
                    for h in range(HPC)]
            psos = pv_state['psos']
            for dt in range(2):
                tt = 2 * (s % 8) + dt
                for h in range(HPC):
                    nc.tensor.matmul(
                        psos[h][0:65, :],
                        lhsT=V_sb[:, b * 16 + tt, h * 65:(h + 1) * 65],
                        rhs=pts[tt][h][:, ts(c, 512)],
                        start=(tt == 0), stop=(tt == 15))
            if s % 8 == 7:
                emit_normalize_half(prev, c)

        # ---- prologue: QT[0:1024] (one [128,1024] psum tile, one staging
        # add) and KT[0:256]; the k-loops follow the 512-column DMA wave
        # order so each matmul starts as soon as its x^T slice lands.
        psQ = psp.tile([128, SC], F32, tag="ps", name="psQ")
        psK = psp.tile([128, SC], F32, tag="ps", name="psK")
        for k in range(8):
            nc.tensor.matmul(psQ[:, 0:512], lhsT=wq_sb[:, k, :],
                             rhs=xT_sb[:, k, 0:512],
                             start=(k == 0), stop=(k == 7))
        for k in range(8):
            nc.tensor.matmul(psK[:, 0:256], lhsT=wk_sb[:, k, :],
                             rhs=xT_sb[:, k, 0:256],
                             start=(k == 0), stop=(k == 7))
        nc.vector.tensor_scalar_add(KT[:, 0:256], psK[:, 0:256], bk_sb)
        for k in range(8):
            nc.tensor.matmul(psQ[:, 512:1024], lhsT=wq_sb[:, k, :],
                             rhs=xT_sb[:, k, 512:1024],
                             start=(k == 0), stop=(k == 7))
        nc.vector.tensor_scalar_add(QT[:, 0:1024], psQ, bq_sb)

        # deferred PE work, interleaved into the ACT-bound attention loop.
        qk = [(wq_sb, bq_sb, QT), (wk_sb, bk_sb, KT)]

        def pj(which, n256):
            return lambda: emit_proj_chunk(*qk[which], n256, w=256)

        def vp(c):
            return lambda: emit_v_proj(c)

        def vt(t):
            return lambda: emit_v_tr(t)

        def wot(t, use_act=False):
            return lambda: emit_wo_tile(t, use_act)

        extras_per_chunk = [
            # chunk 0 (b0,sc0): no PV yet -> heavy. V proj/transposes 0-15
            # (b0 V needed by chunk 1 PV), KT 1-7 just ahead of scores use
            # (tile 2j by step 2j), QT 4-7 (chunk 1), KT 8-9
            [(0, vp(0)), (0, pj(1, 1)), (1, vp(1)), (1, pj(1, 2)),
             (2, vt(0)), (2, vt(1)), (2, vt(2)), (3, pj(1, 3)),
             (3, vt(3)), (4, vt(4)), (4, vt(5)), (5, pj(1, 4)),
             (5, vp(2)), (6, vt(6)), (6, vt(7)), (7, pj(1, 5)),
             (7, vt(8)), (8, vt(9)), (8, vt(10)), (9, pj(1, 6)),
             (9, vt(11)), (10, vp(3)), (11, pj(1, 7)), (11, pj(0, 4)),
             (12, vt(12)), (12, vt(13)), (12, pj(0, 5)), (13, vt(14)),
             (13, pj(0, 6)), (14, vt(15)), (14, pj(0, 7)), (15, pj(1, 8)),
             (15, pj(1, 9))],
            # chunk 1 (b0,sc1): KT 10-15, QT 8-11 (chunk 2), V proj 4 (b1)
            [(0, pj(1, 10)), (2, pj(1, 11)), (4, pj(1, 12)), (5, pj(0, 8)),
             (6, pj(1, 13)), (8, pj(0, 9)), (9, pj(1, 14)), (10, pj(0, 10)),
             (11, pj(1, 15)), (12, pj(0, 11)), (13, vp(4))],
            # chunk 2 (b1,sc0): QT 12-15 (chunk 3), V 5-7 + tr 16-23 (b1,
            # needed by chunk 3 PV), WO 0-3 (chunk 0 tokens, normalized
            # during chunk 1)
            [(0, vt(16)), (0, vt(17)), (1, vt(18)), (1, vt(19)),
             (2, vp(5)), (3, pj(0, 12)), (4, vt(20)), (4, vt(21)),
             (5, vt(22)), (5, vt(23)), (6, pj(0, 13)), (7, vp(6)),
             (8, pj(0, 14)), (9, wot(0)), (10, pj(0, 15)), (11, wot(1)),
             (12, vp(7)), (13, wot(2)), (14, wot(3))],
            # chunk 3 (b1,sc1): V tr 24-31 early (PV of chunk 2 consumes
            # them at steps 4-7), WO 4-15 (chunks 0-1 tokens)
            [(0, vt(24)), (0, vt(25)), (1, vt(26)), (1, vt(27)),
             (2, vt(28)), (2, vt(29)), (3, vt(30)), (3, vt(31)),
             (4, wot(4)), (5, wot(5)), (6, wot(6)), (7, wot(7)),
             (8, wot(8)), (9, wot(9)), (10, wot(10)), (11, wot(11)),
             (12, wot(12)), (13, wot(13)), (14, wot(14)), (15, wot(15))],
        ]

        def emit_scores(b, sc, tt):
            # one t-tile of S^T for both heads -> psum pair; returns the pair
            s0 = b * S + sc * SC
            pair = []
            for h in range(HPC):
                ps = psp.tile([128, SC], F32, tag="ps")
                hsl = slice(h * DK, (h + 1) * DK)
                for n2 in range(SC // 512):
                    nc.tensor.matmul(
                        ps[:, ts(n2, 512)],
                        lhsT=KT[hsl, b * S + tt * 128:b * S + (tt + 1) * 128],
                        rhs=QT[hsl, s0 + n2 * 512:s0 + (n2 + 1) * 512],
                        start=True, stop=True)
                pair.append(ps)
            return pair

        chunks = [(b, sc) for b in range(B) for sc in range(S // SC)]
        prev = None
        pair = emit_scores(0, 0, 0)
        for ci, (b, sc) in enumerate(chunks):
            extras = sorted(extras_per_chunk[ci], key=lambda e: e[0])
            pts = []
            cur = (b, sc, pts, {})
            for tt in range(16):
                row = []
                for h in range(HPC):
                    pt = ptp.tile([128, SC], BF16, tag="pt")
                    nc.scalar.activation(
                        out=pt, in_=pair[h],
                        func=mybir.ActivationFunctionType.Exp,
                        scale=0.125)
                    row.append(pt)
                pts.append(row)
                # emit next scores (crossing chunk boundaries) ahead of the
                # slower PE work so ScalarE's psum slots refill immediately
                if tt + 1 < 16:
                    pair = emit_scores(b, sc, tt + 1)
                elif ci + 1 < len(chunks):
                    pair = emit_scores(*chunks[ci + 1], 0)
                if prev is not None:
                    emit_pv_step(prev, tt)
                while extras and extras[0][0] <= tt:
                    extras.pop(0)[1]()
            for _, e in extras:
                e()
            prev = cur
        # tail: PV + normalize for the last chunk.  WO goes through the
        # now-free scores pool in [128,1024] pairs (4 matmuls in flight
        # instead of 2 -- the 2-slot pse pool serializes WO at ~1.4us/mm).
        def emit_wo_tail(tt):
            # alternate the psp (free after the last scores) and pse pools
            # so 4 WO matmuls are in flight; stage eh1 on ACT via vstg
            for eh in range(2):
                if eh == 0:
                    pwt = psp.tile([128, SC], F32, tag="ps",
                                   name=f"pw{tt}_{eh}")
                    pw = pwt[:, 0:512]
                else:
                    pw = pse.tile([128, 512], F32, tag="pse",
                                  name=f"pw{tt}_{eh}")
                nc.tensor.matmul(pw, lhsT=attnT[:, ts(tt, 128)],
                                 rhs=wo_sb[:, ts(eh, 512)],
                                 start=True, stop=True)
                if eh == 1:
                    ob = vstg.tile([128, 512], BF16, tag="vt",
                                   name=f"obt{tt}")
                    nc.scalar.activation(
                        out=ob, in_=pw,
                        func=mybir.ActivationFunctionType.Copy, bias=0.0)
                else:
                    ob = stg.tile([128, 512], BF16, tag="ob")
                    nc.vector.tensor_copy(ob, pw)
                nc.sync.dma_start(
                    out=out[tt * 128:(tt + 1) * 128, eh * 512:(eh + 1) * 512],
                    in_=ob)

        for s in range(16):
            emit_pv_step(prev, s)
            if s < 8:
                emit_wo_tail(16 + s)
            elif s in (9, 11, 13, 15):
                emit_wo_tail(24 + (s - 9) // 2)
        for tt in range(28, 32):
            emit_wo_tail(tt)


def _prep_in_maps(x, wq, bq, wk, bk, wv, bv, wo):
    x2 = np.asarray(x, np.float32).reshape(NT, D)
    xT = np.ascontiguousarray(x2.T).astype(NPBF16)
    wq = np.asarray(wq, np.float32)
    wk = np.asarray(wk, np.float32)
    wv = np.asarray(wv, np.float32)
    wo = np.asarray(wo, np.float32)
    bq = np.asarray(bq, np.float32)
    bk = np.asarray(bk, np.float32)
    bv = np.asarray(bv, np.float32)

    def wslice(w, cs):
        # [1024, 128] core slice -> [p, k*c] = [128, 1024] contiguous
        wt = w[:, cs].reshape(8, 128, 128).transpose(1, 0, 2)
        return np.ascontiguousarray(wt.reshape(128, D)).astype(NPBF16)

    in_maps = []
    for c in range(NCORES):
        cs = slice(c * 128, (c + 1) * 128)
        in_maps.append({
            "xT": xT,
            "wq": wslice(wq, cs),
            "wk": wslice(wk, cs),
            "wv": wslice(wv, cs),
            "bqkv": np.ascontiguousarray(
                np.stack([bq[cs], bk[cs], bv[cs]], axis=1)),
            "wo": wo[cs, :].astype(NPBF16),
        })
    return in_maps


def kernel(x, wq, bq, wk, bk, wv, bv, wo, bo, _run_kwargs=None):
    if "nc" not in _CACHE:
        _CACHE["nc"] = _build_nc()
    nc = _CACHE["nc"]
    in_maps = _prep_in_maps(x, wq, bq, wk, bk, wv, bv, wo)
    res = run_bass_kernel_spmd(nc, in_maps, list(range(NCORES)),
                               **(_run_kwargs or {}))
    acc = np.zeros((NT, D), np.float32)
    for c in range(NCORES):
        acc += res.results[c]["out"].astype(np.float32)
    acc += np.asarray(bo, np.float32)[None, :]
    if _run_kwargs:
        _CACHE["last_results"] = res
    return acc.reshape(B, S, D)


# revision 24
# speedup vs baseline: 1.1774x; 1.1774x over previous
"""Multi-head self-attention on 8 Trainium2 NeuronCores.

Tensor-parallel over heads: core c owns heads 2c, 2c+1 (128 of the 1024
hidden columns).  The host pre-transposes x to x^T [1024, 4096] bf16 and
the per-core weight slices to [p, k*c] layout so every DMA is contiguous
2KB-per-partition lines.  Stages:
  1. Q^T/K^T = (w.T @ x^T + b) in [d, token] layout (2 heads stacked on
     partitions: 0:64 head0, 64:128 head1).
  2. V^T likewise, then PE transposes into V_aug [token, 65-per-head]
     where column 64/129 = 1.0 (ones column -> softmax denominator falls
     out of P@V as accumulator row 64).
  3. Attention in 4 chunks of (batch, 1024 queries), software-pipelined:
     scores^T tiles = K^T.T @ Q^T (K=64 contraction; the two heads run
     concurrently in disjoint PE row groups), P^T = exp(S^T/8) on ScalarE
     (|S/8| < 3 so exp cannot overflow), and the previous chunk's P@V
     accumulation plus projection/WO back-work fill the PE while ScalarE
     (the bottleneck, ~1.1us per [128,1024] exp) streams.
  4. normalize: rsum = recip(pso row 64) straight from PSUM, gpsimd
     partition_broadcast, one fused tensor_mul PSUM->attnT (bf16).
  5. partial = attnT.T @ wo[128 rows of this core] -> HBM (bf16), the
     two 512-col halves staged into one [128,1024] tile, single DMA.
Host sums the 8 partials and adds bo.

Scheduling: ScalarE must never starve.  Warm-up matmuls + the exp
ACT-table load issue at t=0 with no DMA dependency (HAM un-throttles the
PE clock during the DMA window and the 2.7us table load is off the
critical path).  x^T lands in first-use order; weights go on the
tensor/vector DMA queues so they never queue behind x^T.  The extras
(projection chunks, V build, WO tiles) are spread so each chunk's PE
work stays under the ~35.7us exp stream; scores for step tt+1 (and for
the next chunk's tt=0 at boundaries) are emitted before slower PE work.

Shapes hardcoded for x:[2,2048,1024], 16 heads, d_k=64.
"""

import numpy as np
import ml_dtypes

import concourse.bass as bass
import concourse.tile as tile
from concourse import bacc, mybir
from concourse.bass import ts
from concourse.bass_utils import run_bass_kernel_spmd

BF16 = mybir.dt.bfloat16
F32 = mybir.dt.float32
NPBF16 = ml_dtypes.bfloat16

B = 2
S = 2048
D = 1024
NT = B * S  # 4096 tokens
DK = 64
NCORES = 8
HPC = 2  # heads per core
SC = 1024  # attention s-chunk (exp op free size)

_CACHE = {}


def _build_nc():
    nc = bacc.Bacc("TRN2", target_bir_lowering=False, debug=False,
                   num_devices=NCORES)

    xT = nc.dram_tensor("xT", [D, NT], BF16, kind="ExternalInput").ap()
    # weights host-transposed to [p, k*128] so the DMA is contiguous
    wq = nc.dram_tensor("wq", [128, D], BF16, kind="ExternalInput").ap()
    wk = nc.dram_tensor("wk", [128, D], BF16, kind="ExternalInput").ap()
    wv = nc.dram_tensor("wv", [128, D], BF16, kind="ExternalInput").ap()
    bqkv = nc.dram_tensor("bqkv", [128, 3], F32, kind="ExternalInput").ap()
    wo = nc.dram_tensor("wo", [128, D], BF16, kind="ExternalInput").ap()
    out = nc.dram_tensor("out", [NT, D], BF16, kind="ExternalOutput").ap()

    with tile.TileContext(nc) as tc:
        _emit(nc, tc, xT, wq, wk, wv, bqkv, wo, out)
    nc.compile()
    return nc


def _emit(nc, tc, xT, wq, wk, wv, bqkv, wo, out):
    import contextlib
    ctx = contextlib.ExitStack()
    with ctx:
        consts = ctx.enter_context(tc.tile_pool(name="consts", bufs=1))
        ptp = ctx.enter_context(tc.tile_pool(name="ptp", bufs=46))
        psp = ctx.enter_context(tc.tile_pool(name="psp", bufs=2, space="PSUM"))
        pvp = ctx.enter_context(tc.tile_pool(name="pvp", bufs=2, space="PSUM"))
        pse = ctx.enter_context(tc.tile_pool(name="pse", bufs=2, space="PSUM"))
        stg = ctx.enter_context(tc.tile_pool(name="stg", bufs=2))
        nrm = ctx.enter_context(tc.tile_pool(name="nrm", bufs=2))
        vstg = ctx.enter_context(tc.tile_pool(name="vstg", bufs=2))

        # ---- persistent SBUF tensors ----
        xT_sb = consts.tile([128, 8, NT], BF16)      # 8 k-tiles of x^T
        wq_sb = consts.tile([128, 8, 128], BF16)
        wk_sb = consts.tile([128, 8, 128], BF16)
        wv_sb = consts.tile([128, 8, 128], BF16)
        bqkv_sb = consts.tile([128, 3], F32)
        bq_sb = bqkv_sb[:, 0:1]
        bk_sb = bqkv_sb[:, 1:2]
        bv_sb = bqkv_sb[:, 2:3]
        wo_sb = consts.tile([128, D], BF16)
        QT = consts.tile([128, NT], BF16)
        KT = consts.tile([128, NT], BF16)
        V_sb = consts.tile([128, 32, 130], BF16)     # [t-in-tile, t_tile, col]
        attnT = consts.tile([128, NT], BF16)
        ident = consts.tile([128, 128], BF16)
        wup = consts.tile([128, 128], BF16)

        # ---- t=0: PE warm-up + ACT table load, no DMA dependencies ----
        nc.vector.memset(wup, 0.125)
        wups = pse.tile([128, 512], F32, tag="pse", name="wups")
        for i in range(64):
            nc.tensor.matmul(wups[:, 0:128], lhsT=wup, rhs=wup,
                             start=True, stop=True)
        tblw = stg.tile([128, 1], F32, tag="tblw", name="tblw")
        nc.scalar.activation(out=tblw, in_=wup[:, 0:1],
                             func=mybir.ActivationFunctionType.Exp)
        from concourse.masks import make_identity
        make_identity(nc, ident)

        # ---- DMAs: weights first on the gpsimd queue; x^T in merged
        # multi-k-tile waves (one dma_start covers all 8 k-tiles of a
        # column range -- 8x fewer descriptor-gen instructions).  The
        # scalar (ACT) queue carries exactly one wave so the exp stream
        # is never blocked behind descriptor generation. ----
        xT_p = xT.rearrange("(k p) n -> p k n", p=128)

        # critical first: x^T[:, 0:1024] (2MB) split across the sync and
        # scalar queues, weights in parallel on gpsimd.  The rest follows
        # once the critical slices are in flight.
        # the critical x^T[:, 0:1024] is split across ALL THREE queues
        # (per-queue bandwidth is only ~130 B/ns); small weights slot in
        # by first-need time.
        nc.sync.dma_start(out=xT_sb[:, 0:3, 0:1024], in_=xT_p[:, 0:3, 0:1024])
        nc.scalar.dma_start(out=xT_sb[:, 3:6, 0:1024],
                            in_=xT_p[:, 3:6, 0:1024])
        nc.gpsimd.dma_start(out=xT_sb[:, 6:8, 0:1024],
                            in_=xT_p[:, 6:8, 0:1024])
        nc.gpsimd.dma_start(out=wq_sb, in_=wq.rearrange("p (k c) -> p k c", k=8))
        nc.gpsimd.dma_start(out=bqkv_sb, in_=bqkv)
        nc.gpsimd.dma_start(out=wk_sb, in_=wk.rearrange("p (k c) -> p k c", k=8))
        nc.scalar.dma_start(out=wv_sb, in_=wv.rearrange("p (k c) -> p k c", k=8))
        nc.sync.dma_start(out=xT_sb[:, :, 1024:1536],
                          in_=xT_p[:, :, 1024:1536])
        nc.gpsimd.dma_start(out=xT_sb[:, :, 1536:2048],
                            in_=xT_p[:, :, 1536:2048])
        nc.sync.dma_start(out=xT_sb[:, :, 2048:3072],
                          in_=xT_p[:, :, 2048:3072])
        nc.gpsimd.dma_start(out=wo_sb, in_=wo)
        nc.gpsimd.dma_start(out=xT_sb[:, :, 3072:4096],
                            in_=xT_p[:, :, 3072:4096])

        # ones columns of V_aug (never touched by the per-tile copies)
        nc.vector.memset(V_sb[:, :, 64:65], 1.0)
        nc.vector.memset(V_sb[:, :, 129:130], 1.0)

        # ---- emit helpers ----
        vt_stage = {}

        def emit_v_proj(c):
            # V^T chunk: [c128, 512 tokens] += wv[k].T @ xT[k] (+bias, ->bf16)
            psv = pse.tile([128, 512], F32, tag="pse")
            for k in range(8):
                nc.tensor.matmul(psv, lhsT=wv_sb[:, k, :],
                                 rhs=xT_sb[:, k, ts(c, 512)],
                                 start=(k == 0), stop=(k == 7))
            vt = vstg.tile([128, 512], BF16, tag="vt", name=f"vt{c}")
            nc.vector.tensor_scalar_add(vt, psv, bv_sb)
            vt_stage[c] = vt

        def emit_v_tr(tt):
            # transpose one 128x128 block of V^T into V_aug [t, col] layout
            c, j = divmod(tt, 4)
            trp = pse.tile([128, 128], BF16, tag="pse", name=f"trp{tt}")
            nc.tensor.transpose(trp, vt_stage[c][:, ts(j, 128)], ident)
            nc.vector.tensor_copy(V_sb[:, tt, 0:64], trp[:, 0:64])
            nc.vector.tensor_copy(V_sb[:, tt, 65:129], trp[:, 64:128])

        def emit_wo_tile(tt, use_act=False):
            for eh in range(2):
                pw = pse.tile([128, 512], F32, tag="pse")
                nc.tensor.matmul(pw, lhsT=attnT[:, ts(tt, 128)],
                                 rhs=wo_sb[:, ts(eh, 512)],
                                 start=True, stop=True)
                ob = stg.tile([128, 512], BF16, tag="ob")
                if use_act and eh == 1:
                    nc.scalar.activation(
                        out=ob, in_=pw,
                        func=mybir.ActivationFunctionType.Copy, bias=0.0)
                else:
                    nc.vector.tensor_copy(ob, pw)
                nc.sync.dma_start(
                    out=out[tt * 128:(tt + 1) * 128, eh * 512:(eh + 1) * 512],
                    in_=ob)

        def emit_proj_chunk(w_sb, b_sb, o_sb, n, w=512):
            # w-token chunk n (units of w) of the Q^T or K^T projection
            ps = pse.tile([128, 512], F32, tag="pse")
            for k in range(8):
                nc.tensor.matmul(ps[:, 0:w], lhsT=w_sb[:, k, :],
                                 rhs=xT_sb[:, k, ts(n, w)],
                                 start=(k == 0), stop=(k == 7))
            nc.vector.tensor_scalar_add(o_sb[:, ts(n, w)], ps[:, 0:w], b_sb)

        def emit_normalize_half(prev, c):
            # stage the accumulator out of PSUM first (one copy frees the
            # bank for the next PV half).  reciprocal_approx_fast is a
            # custom-DVE op: it needs a partition-0 SBUF input (PSUM or
            # offset-64 inputs produce garbage / hw crashes).
            b, sc, pts, pv_state = prev
            s0 = b * S + sc * SC + c * 512
            for h in range(HPC):
                pso = pv_state['psos'][h]
                ostg = nrm.tile([65, 512], F32, tag="ostg", bufs=1)
                nc.vector.tensor_copy(ostg, pso[0:65, :])
                rsum = nrm.tile([1, 512], F32, tag="rsum", bufs=1)
                nc.vector.tensor_copy(rsum, ostg[64:65, :])
                nc.vector.reciprocal_approx_fast(out=rsum, in_=rsum)
                recb = nrm.tile([64, 512], F32, tag="recb", bufs=1)
                nc.gpsimd.partition_broadcast(recb, rsum)
                nc.vector.tensor_mul(
                    attnT[h * DK:(h + 1) * DK, s0:s0 + 512],
                    ostg[0:64, :], recb)

        def emit_pv_step(prev, s):
            # one pipeline step of P@V for the previous chunk: two t-tiles
            # into the [65, 512] accumulators of half-chunk c = s // 8
            b, sc, pts, pv_state = prev
            c = s // 8
            if s % 8 == 0:
                pv_state['psos'] = [
                    pvp.tile([128, 512], F32, tag="pv",
                             name=f"pso{b}_{sc}_{c}_{h}")
                    for h in range(HPC)]
            psos = pv_state['psos']
            for dt in range(2):
                tt = 2 * (s % 8) + dt
                for h in range(HPC):
                    nc.tensor.matmul(
                        psos[h][0:65, :],
                        lhsT=V_sb[:, b * 16 + tt, h * 65:(h + 1) * 65],
                        rhs=pts[tt][h][:, ts(c, 512)],
                        start=(tt == 0), stop=(tt == 15))
            if s % 8 == 7:
                emit_normalize_half(prev, c)

        # ---- prologue: QT[0:1024] (one [128,1024] psum tile, one staging
        # add) and KT[0:256]; the k-loops follow the 512-column DMA wave
        # order so each matmul starts as soon as its x^T slice lands.
        psQ = psp.tile([128, SC], F32, tag="ps", name="psQ")
        psK = psp.tile([128, SC], F32, tag="ps", name="psK")
        for k in range(8):
            nc.tensor.matmul(psQ[:, 0:512], lhsT=wq_sb[:, k, :],
                             rhs=xT_sb[:, k, 0:512],
                             start=(k == 0), stop=(k == 7))
        for k in range(8):
            nc.tensor.matmul(psK[:, 0:256], lhsT=wk_sb[:, k, :],
                             rhs=xT_sb[:, k, 0:256],
                             start=(k == 0), stop=(k == 7))
        nc.vector.tensor_scalar_add(KT[:, 0:256], psK[:, 0:256], bk_sb)
        for k in range(8):
            nc.tensor.matmul(psQ[:, 512:1024], lhsT=wq_sb[:, k, :],
                             rhs=xT_sb[:, k, 512:1024],
                             start=(k == 0), stop=(k == 7))
        nc.vector.tensor_scalar_add(QT[:, 0:1024], psQ, bq_sb)

        # deferred PE work, interleaved into the ACT-bound attention loop.
        qk = [(wq_sb, bq_sb, QT), (wk_sb, bk_sb, KT)]

        def pj(which, n256):
            return lambda: emit_proj_chunk(*qk[which], n256, w=256)

        def vp(c):
            return lambda: emit_v_proj(c)

        def vt(t):
            return lambda: emit_v_tr(t)

        def wot(t, use_act=False):
            return lambda: emit_wo_tile(t, use_act)

        extras_per_chunk = [
            # chunk 0 (b0,sc0): no PV yet -> heavy. V proj/transposes 0-15
            # (b0 V needed by chunk 1 PV), KT 1-7 just ahead of scores use
            # (tile 2j by step 2j), QT 4-7 (chunk 1), KT 8-9
            [(0, vp(0)), (0, pj(1, 1)), (1, vp(1)), (1, pj(1, 2)),
             (2, vt(0)), (2, vt(1)), (2, vt(2)), (3, pj(1, 3)),
             (3, vt(3)), (4, vt(4)), (4, vt(5)), (5, pj(1, 4)),
             (5, vp(2)), (6, vt(6)), (6, vt(7)), (7, pj(1, 5)),
             (7, vt(8)), (8, vt(9)), (8, vt(10)), (9, pj(1, 6)),
             (9, vt(11)), (10, vp(3)), (11, pj(1, 7)), (11, pj(0, 4)),
             (12, vt(12)), (12, vt(13)), (12, pj(0, 5)), (13, vt(14)),
             (13, pj(0, 6)), (14, vt(15)), (14, pj(0, 7)), (15, pj(1, 8)),
             (15, pj(1, 9))],
            # chunk 1 (b0,sc1): KT 10-15, QT 8-11 (chunk 2), V proj 4 (b1)
            [(0, pj(1, 10)), (2, pj(1, 11)), (4, pj(1, 12)), (5, pj(0, 8)),
             (6, pj(1, 13)), (8, pj(0, 9)), (9, pj(1, 14)), (10, pj(0, 10)),
             (11, pj(1, 15)), (12, pj(0, 11)), (13, vp(4))],
            # chunk 2 (b1,sc0): QT 12-15 (chunk 3), V 5-7 + tr 16-23 (b1,
            # needed by chunk 3 PV), WO 0-3 (chunk 0 tokens, normalized
            # during chunk 1)
            [(0, vt(16)), (0, vt(17)), (1, vt(18)), (1, vt(19)),
             (2, vp(5)), (3, pj(0, 12)), (4, vt(20)), (4, vt(21)),
             (5, vt(22)), (5, vt(23)), (6, pj(0, 13)), (7, vp(6)),
             (8, pj(0, 14)), (9, wot(0)), (10, pj(0, 15)), (11, wot(1)),
             (12, vp(7)), (13, wot(2)), (14, wot(3))],
            # chunk 3 (b1,sc1): V tr 24-31 early (PV of chunk 2 consumes
            # them at steps 4-7), WO 4-15 (chunks 0-1 tokens)
            [(0, vt(24)), (0, vt(25)), (1, vt(26)), (1, vt(27)),
             (2, vt(28)), (2, vt(29)), (3, vt(30)), (3, vt(31)),
             (4, wot(4)), (5, wot(5)), (6, wot(6)), (7, wot(7)),
             (8, wot(8)), (9, wot(9)), (10, wot(10)), (11, wot(11)),
             (12, wot(12)), (13, wot(13)), (14, wot(14)), (15, wot(15))],
        ]

        def emit_scores_head(b, sc, tt, h):
            # one head's t-tile of S^T -> one psum tile
            s0 = b * S + sc * SC
            ps = psp.tile([128, SC], F32, tag="ps")
            hsl = slice(h * DK, (h + 1) * DK)
            for n2 in range(SC // 512):
                nc.tensor.matmul(
                    ps[:, ts(n2, 512)],
                    lhsT=KT[hsl, b * S + tt * 128:b * S + (tt + 1) * 128],
                    rhs=QT[hsl, s0 + n2 * 512:s0 + (n2 + 1) * 512],
                    start=True, stop=True)
            return ps

        def emit_scores(b, sc, tt):
            return [emit_scores_head(b, sc, tt, h) for h in range(HPC)]

        chunks = [(b, sc) for b in range(B) for sc in range(S // SC)]
        prev = None
        pair = emit_scores(0, 0, 0)
        for ci, (b, sc) in enumerate(chunks):
            extras = sorted(extras_per_chunk[ci], key=lambda e: e[0])
            pts = []
            cur = (b, sc, pts, {})
            for tt in range(16):
                # next tile to prefill (crossing chunk boundaries)
                if tt + 1 < 16:
                    nxt = (b, sc, tt + 1)
                elif ci + 1 < len(chunks):
                    nxt = (*chunks[ci + 1], 0)
                else:
                    nxt = None
                row = []
                npair = []
                for h in range(HPC):
                    pt = ptp.tile([128, SC], BF16, tag="pt")
                    nc.scalar.activation(
                        out=pt, in_=pair[h],
                        func=mybir.ActivationFunctionType.Exp,
                        scale=0.125)
                    row.append(pt)
                    # refill this head's psum slot DURING the other head's
                    # exp -- the slot frees the moment exp(h) completes
                    if nxt is not None:
                        npair.append(emit_scores_head(*nxt, h))
                pts.append(row)
                pair = npair
                if prev is not None:
                    emit_pv_step(prev, tt)
                while extras and extras[0][0] <= tt:
                    extras.pop(0)[1]()
            for _, e in extras:
                e()
            prev = cur
        # tail: PV + normalize for the last chunk.  WO goes through the
        # now-free scores pool in [128,1024] pairs (4 matmuls in flight
        # instead of 2 -- the 2-slot pse pool serializes WO at ~1.4us/mm).
        def emit_wo_tail(tt):
            # alternate the psp (free after the last scores) and pse pools
            # so 4 WO matmuls are in flight; stage eh1 on ACT via vstg
            for eh in range(2):
                if eh == 0:
                    pwt = psp.tile([128, SC], F32, tag="ps",
                                   name=f"pw{tt}_{eh}")
                    pw = pwt[:, 0:512]
                else:
                    pw = pse.tile([128, 512], F32, tag="pse",
                                  name=f"pw{tt}_{eh}")
                nc.tensor.matmul(pw, lhsT=attnT[:, ts(tt, 128)],
                                 rhs=wo_sb[:, ts(eh, 512)],
                                 start=True, stop=True)
                if eh == 1:
                    ob = vstg.tile([128, 512], BF16, tag="vt",
                                   name=f"obt{tt}")
                    nc.scalar.activation(
                        out=ob, in_=pw,
                        func=mybir.ActivationFunctionType.Copy, bias=0.0)
                else:
                    ob = stg.tile([128, 512], BF16, tag="ob")
                    nc.vector.tensor_copy(ob, pw)
                nc.sync.dma_start(
                    out=out[tt * 128:(tt + 1) * 128, eh * 512:(eh + 1) * 512],
                    in_=ob)

        for s in range(16):
            emit_pv_step(prev, s)
            if s < 8:
                emit_wo_tail(16 + s)
            elif s in (9, 11, 13, 15):
                emit_wo_tail(24 + (s - 9) // 2)
        for tt in range(28, 32):
            emit_wo_tail(tt)


def _prep_in_maps(x, wq, bq, wk, bk, wv, bv, wo):
    x2 = np.asarray(x, np.float32).reshape(NT, D)
    xT = np.ascontiguousarray(x2.T).astype(NPBF16)
    wq = np.asarray(wq, np.float32)
    wk = np.asarray(wk, np.float32)
    wv = np.asarray(wv, np.float32)
    wo = np.asarray(wo, np.float32)
    bq = np.asarray(bq, np.float32)
    bk = np.asarray(bk, np.float32)
    bv = np.asarray(bv, np.float32)

    def wslice(w, cs):
        # [1024, 128] core slice -> [p, k*c] = [128, 1024] contiguous
        wt = w[:, cs].reshape(8, 128, 128).transpose(1, 0, 2)
        return np.ascontiguousarray(wt.reshape(128, D)).astype(NPBF16)

    in_maps = []
    for c in range(NCORES):
        cs = slice(c * 128, (c + 1) * 128)
        in_maps.append({
            "xT": xT,
            "wq": wslice(wq, cs),
            "wk": wslice(wk, cs),
            "wv": wslice(wv, cs),
            "bqkv": np.ascontiguousarray(
                np.stack([bq[cs], bk[cs], bv[cs]], axis=1)),
            "wo": wo[cs, :].astype(NPBF16),
        })
    return in_maps


def kernel(x, wq, bq, wk, bk, wv, bv, wo, bo, _run_kwargs=None):
    if "nc" not in _CACHE:
        _CACHE["nc"] = _build_nc()
    nc = _CACHE["nc"]
    in_maps = _prep_in_maps(x, wq, bq, wk, bk, wv, bv, wo)
    res = run_bass_kernel_spmd(nc, in_maps, list(range(NCORES)),
                               **(_run_kwargs or {}))
    acc = np.zeros((NT, D), np.float32)
    for c in range(NCORES):
        acc += res.results[c]["out"].astype(np.float32)
    acc += np.asarray(bo, np.float32)[None, :]
    if _run_kwargs:
        _CACHE["last_results"] = res
    return acc.reshape(B, S, D)
